# revision 17
# baseline (speedup 1.0000x reference)
"""Binarized ResNet Bottleneck block (sign-binarized convs + BN + residual)
for Trainium2, data-parallel over 8 NeuronCores (8 images per core).

Math (per reference):
  out1 = BN1(conv1x1(sign(x),  sign(w1)))        # 1024 -> 256
  out2 = BN2(conv3x3(sign(out1), sign(w2)))      # 256 -> 256, pad 1
  out3 = BN3(conv1x1(sign(out2), sign(w3)))      # 256 -> 1024
  y    = out3 + x
(htanh's only feed sign() and sign(htanh(t)) == sign(t), so they drop.)

v2 design (per-core, 8 images as 2 super-blocks of 4):
  - sign(x) is precomputed on the HOST and shipped as fp8e4 (+-1 exact);
    the fp32 x is shipped as bf16 only for the residual add. Output is
    written back as bf16 (rel-err ~2e-3 << 2e-2 tolerance).
  - convs are fp8 DoubleRow matmuls (K=256/matmul). HW-measured pacing is
    N/2.4GHz per matmul with LDWEIGHTS fully hidden, so the loops simply
    keep N large (392 for conv1/conv3, 196 windows for conv2) and
    back-to-back: ~23us of PE work is the kernel's critical path.
  - one shared PSUM pool of 4x [128,2,512] tiles (8 banks) rotates through
    conv1/conv2/conv3 uses; conv3 drains overlap its own matmul stream.
  - BN1/BN2+sign: one ACT op per (m[,gc]) straight into the zero-padded
    conv2 input planes / flat conv3 input.
  - BN3+residual: even output tiles use the fused DVE custom op
    AFFINE_THEN_ADD (out = ps*sc + sh + x, one pass); odd tiles use ACT
    (affine, PSUM->bf16) + an in-place bf16 add (GpSimd for SB0, DVE for
    SB1) so the three engines share the drain work.
  - 8 dummy matmuls on uninitialized SBUF warm the PE HAM clock-gate
    during the initial DMA runway.
"""

import numpy as np
import ml_dtypes

N_CORES = 8
B = 64              # global batch
CIN = 1024
P = 256             # bottleneck width
NPX = 196           # 14*14
NSB = 2             # super-blocks per core
BSB = 4             # images per super-block
COLS = BSB * NPX    # 784 moving columns per super-block
HCOL = COLS // 2    # 392, columns per matmul (fits a PSUM bank)

_EPS = 1e-5

_state = {}


def _build_nc():
    import concourse.bass as bass
    import concourse.mybir as mybir
    from concourse import bacc
    from concourse.tile import TileContext

    fp32 = mybir.dt.float32
    bf16 = mybir.dt.bfloat16
    f8 = mybir.dt.float8e4
    DR = mybir.MatmulPerfMode.DoubleRow
    SIGN = mybir.ActivationFunctionType.Sign
    COPY = mybir.ActivationFunctionType.Copy
    IDENT = mybir.ActivationFunctionType.Identity
    MULT = mybir.AluOpType.mult
    ADD = mybir.AluOpType.add
    GE = mybir.AluOpType.is_ge
    SUB = mybir.AluOpType.subtract

    nc = bacc.Bacc(None, target_bir_lowering=False)

    # host-binarized sign(x): channel (2t+k)*128+p, col = img*196+px
    xs = nc.dram_tensor("xs", [NSB, 128, 4, 2, COLS], f8, kind="ExternalInput")
    # residual x in bf16: channel m*128+p (m = 2t+k, same order), col as xs
    xr = nc.dram_tensor("xr", [NSB, 128, 8, COLS], bf16, kind="ExternalInput")
    # binarized fp8 weights, DoubleRow-interleaved (see _prep_inputs):
    # cols [0:2048]=w1 (4t x 2k x 256), [2048:6656]=w2 (9 tap x 2k x 256),
    # [6656:8704]=w3 (2k x 1024)
    wb = nc.dram_tensor("wb", [128, 8704], f8, kind="ExternalInput")
    # BN params: sc1(2) sh1(2) sc2(2) sh2(2) sc3(8) sh3(8)
    bnp = nc.dram_tensor("bnp", [128, 26], fp32, kind="ExternalInput")
    yt = nc.dram_tensor("yt", [NSB, 128, 8, COLS], bf16, kind="ExternalOutput")

    with TileContext(nc) as tc:
        with (
            tc.tile_pool(name="consts", bufs=1) as cpool,
            tc.tile_pool(name="xb3_pool", bufs=2) as xb3_pool,
            tc.tile_pool(name="out_pool", bufs=2) as out_pool,
            tc.tile_pool(name="ps_pool", bufs=4, space="PSUM") as ps_pool,
        ):
            wb_sb = cpool.tile([128, 8704], f8, name="wb_sb")
            w1_sb = wb_sb[:, 0:2048].rearrange("p (t k c) -> p t k c", t=4, k=2)
            w2_sb = wb_sb[:, 2048:6656].rearrange("p (t k c) -> p t k c", t=9, k=2)
            w3_sb = wb_sb[:, 6656:8704].rearrange("p (k c) -> p k c", k=2)

            bnp_sb = cpool.tile([128, 26], fp32, name="bnp_sb")
            sc1_sb = bnp_sb[:, 0:2]
            sh1_sb = bnp_sb[:, 2:4]
            sc2_sb = bnp_sb[:, 4:6]
            sh2_sb = bnp_sb[:, 6:8]
            sc3_sb = bnp_sb[:, 8:16]
            sh3_sb = bnp_sb[:, 16:24]
            thr1_sb = bnp_sb[:, 24:25]   # -sh1/sc1, m=1 half
            thr2_sb = bnp_sb[:, 25:26]   # -sh2/sc2, m=1 half

            # ---- PE warmup: dummy matmuls keep the HAM clock-gate busy
            # during the input-DMA runway.
            dum_w = cpool.tile([128, 2, 128], f8, name="dum_w")
            dum_x = cpool.tile([128, 2, HCOL], f8, name="dum_x")
            nc.vector.memset(dum_w, 0.0)
            nc.vector.memset(dum_x, 0.0)
            ps_dum = ps_pool.tile([128, 2, 512], fp32, name="ps", tag="ps")
            for i in range(7):
                nc.tensor.matmul(
                    ps_dum[:, i % 2, :HCOL], dum_w, dum_x,
                    start=True, stop=True, perf_mode=DR,
                    skip_group_check=True,
                )

            xs_t = []
            xr_t = []
            for s in range(NSB):
                xs_t.append(cpool.tile([128, 4, 2, COLS], f8, name=f"xs{s}"))
                xr_t.append(cpool.tile([128, 8, COLS], bf16, name=f"xr{s}"))

            # ---- input DMAs (sync queue), most-needed first -------------
            nc.sync.dma_start(xs_t[0][:, 0:2], xs[0, :, 0:2])
            nc.sync.dma_start(wb_sb[:, 0:512], wb[:, 0:512])       # w1 t0
            nc.sync.dma_start(wb_sb[:, 512:2048], wb[:, 512:2048])  # w1 t1-3
            nc.sync.dma_start(bnp_sb, bnp[:])
            nc.sync.dma_start(xs_t[0][:, 2:4], xs[0, :, 2:4])
            nc.sync.dma_start(wb_sb[:, 2048:8704], wb[:, 2048:8704])  # w2,w3
            nc.sync.dma_start(xr_t[0], xr[0])
            nc.sync.dma_start(xs_t[1], xs[1])
            nc.sync.dma_start(xr_t[1], xr[1])

            # observer ops: make ACT/DVE see the const DMAs once, so later
            # single-wait ISA structs don't need extra event sems.
            scr_a = cpool.tile([128, 26], fp32, name="scr_a")
            nc.scalar.activation(scr_a, bnp_sb, COPY)
            scr_v = cpool.tile([128, 26], fp32, name="scr_v")
            nc.vector.tensor_tensor(scr_v, bnp_sb, bnp_sb, MULT)
            nc.tensor.ldweights(wb_sb[:, 0:128])

            # zero-padded conv2 input planes: [128, 2k, 4img, 16x16]
            xpads = []
            for s in range(NSB):
                xp = cpool.tile([128, 2, BSB, 256], f8, name=f"xpad{s}")
                nc.gpsimd.memset(xp, 0.0)
                xpads.append(xp)

            xb3_t = [
                xb3_pool.tile([128, 2, COLS], f8, name="xb3", tag="xb3")
                for s in range(NSB)
            ]
            out_t = [
                out_pool.tile([128, 8, COLS], bf16, name="out", tag="out")
                for s in range(NSB)
            ]

            def conv1(s):
                """1x1, 1024->256; m-major so BN1(m0) overlaps m1 matmuls."""
                xs_s, xpad = xs_t[s], xpads[s]
                for m in range(2):
                    ps1 = ps_pool.tile([128, 2, 512], fp32, name="ps",
                                       tag="ps")
                    for t in range(4):
                        for gc in range(2):
                            nc.tensor.matmul(
                                ps1[:, gc, :HCOL],
                                w1_sb[:, t, :, m * 128:(m + 1) * 128],
                                xs_s[:, t, :, gc * HCOL:(gc + 1) * HCOL],
                                start=(t == 0), stop=(t == 3),
                                perf_mode=DR, skip_group_check=True,
                            )
                    # BN1+sign into padded planes (imgs 2gc,2gc+1 of
                    # row m). m0 on ACT as Sign (+-1); m1 on DVE as
                    # (ps >= thr) - 0.5 (+-0.5; that half of w2 is doubled
                    # on the host), so both engines share the boundary.
                    for gc in range(2):
                        dst = xpad[:, m, 2 * gc:2 * gc + 2].rearrange(
                            "p b (h w) -> p b h w", h=16
                        )[:, :, 1:15, 1:15]
                        srcp = ps1[:, gc, :HCOL].rearrange(
                            "p (b h w) -> p b h w", b=2, h=14
                        )
                        if m == 0:
                            nc.scalar.activation(
                                dst, srcp, SIGN,
                                bias=sh1_sb[:, 0:1], scale=sc1_sb[:, 0:1],
                            )
                        else:
                            nc.vector.tensor_scalar(
                                dst, srcp, thr1_sb, 0.5, GE, SUB
                            )

            def conv2(s):
                """3x3 pad 1, 256->256; one image per PSUM bank."""
                xpad, xb3 = xpads[s], xb3_t[s]
                ps2 = [
                    [
                        ps_pool.tile([128, 2, 512], fp32, name="ps", tag="ps")
                        for pr in range(2)
                    ]
                    for m in range(2)
                ]
                xpv = xpad.rearrange("p k b (h w) -> p k b h w", h=16)

                def bn2(m, pr):
                    # BN2+sign -> flat conv3 input. m0 on ACT (+-1); m1 on
                    # DVE (+-0.5, that half of w3 doubled on the host).
                    dst = xb3[:, m, pr * HCOL:(pr + 1) * HCOL].rearrange(
                        "p (b n) -> p b n", b=2
                    )
                    srcp = ps2[m][pr][:, :, :NPX]
                    if m == 0:
                        nc.scalar.activation(
                            dst, srcp, SIGN,
                            bias=sh2_sb[:, 0:1], scale=sc2_sb[:, 0:1],
                        )
                    else:
                        nc.vector.tensor_scalar(dst, srcp, thr2_sb, 0.5,
                                                GE, SUB)

                for tap in range(9):
                    ky, kx = tap // 3, tap % 3
                    for m in range(2):
                        wsl = w2_sb[:, tap, :, m * 128:(m + 1) * 128]
                        for b in range(BSB):
                            nc.tensor.matmul(
                                ps2[m][b // 2][:, b % 2, :NPX],
                                wsl,
                                xpv[:, :, b, ky:ky + 14, kx:kx + 14],
                                start=(tap == 0), stop=(tap == 8),
                                perf_mode=DR, skip_group_check=True,
                            )
                            if tap == 8 and b % 2 == 1:
                                bn2(m, b // 2)

            def conv3(s, m_list, dassign):
                """1x1, 256->1024 + BN3-scale + residual (sh3 folded into
                xr on host). dassign[m]: 'dve' = fused DVE custom op;
                'act_pool'/'act_dve' = ACT scale + in-place bf16 add."""
                xb3, xr_s, out_sb = xb3_t[s], xr_t[s], out_t[s]
                for m in m_list:
                    ps3 = ps_pool.tile([128, 2, 512], fp32, name="ps",
                                       tag="ps")
                    for gc in range(2):
                        nc.tensor.matmul(
                            ps3[:, gc, :HCOL],
                            w3_sb[:, :, m * 128:(m + 1) * 128],
                            xb3[:, :, gc * HCOL:(gc + 1) * HCOL],
                            start=True, stop=True,
                            perf_mode=DR, skip_group_check=True,
                        )
                    o_m = out_sb[:, m].rearrange("p (g n) -> p g n", g=2)
                    x_m = xr_s[:, m].rearrange("p (g n) -> p g n", g=2)
                    mode = dassign[m]
                    if mode == "dve":
                        nc.vector.affine_then_add(
                            o_m, ps3[:, :, :HCOL], x_m,
                            sc3_sb[:, m:m + 1], 0.0,
                        )
                    else:
                        nc.scalar.activation(
                            o_m, ps3[:, :, :HCOL], IDENT,
                            scale=sc3_sb[:, m:m + 1],
                        )
                        eng = nc.gpsimd if mode == "act_pool" else nc.vector
                        eng.tensor_tensor(o_m, o_m, x_m, ADD)
                    if m % 2 == 1:
                        mp = m // 2
                        nc.sync.dma_start(
                            yt[s, :, 2 * mp:2 * mp + 2],
                            out_sb[:, 2 * mp:2 * mp + 2],
                        )

            DA0 = {0: "dve", 1: "act_dve", 2: "dve", 3: "act_dve",
                   4: "dve", 5: "act_dve", 6: "dve", 7: "act_dve"}
            DA1 = {0: "dve", 1: "act_dve", 2: "dve", 3: "act_dve",
                   4: "dve", 5: "act_dve", 6: "act_dve", 7: "dve"}

            conv1(0)
            conv2(0)
            conv3(0, [0, 1, 2, 3], DA0)
            conv1(1)                      # fills conv3(0)'s drain window
            conv3(0, [4, 5, 6, 7], DA0)
            conv2(1)
            conv3(1, [0, 1, 2, 3, 4, 5, 6, 7], DA1)

    nc.compile()
    return nc


def _bn_params(g, b, m, v):
    """scale/shift computed with the same jax expressions as the reference."""
    import jax
    import jax.numpy as jnp
    from jax import lax

    ge, be, me, ve = (jnp.asarray(t) for t in (g, b, m, v))
    scale = ge * lax.rsqrt(ve + _EPS)
    shift = be - ge * me * lax.rsqrt(ve + _EPS)
    return np.asarray(scale, np.float32), np.asarray(shift, np.float32)


def _prep_inputs(inputs):
    """Host-side prep: shard batch, binarize weights AND activations,
    fold BN params, bf16 residual copy."""
    f8 = ml_dtypes.float8_e4m3
    bf = ml_dtypes.bfloat16
    x = np.ascontiguousarray(np.asarray(inputs["x"], np.float32))

    # weights -> sign -> fp8e4, DoubleRow-interleaved [128ki, ..., 2k, cout]
    w1 = np.sign(np.asarray(inputs["w1"], np.float32)[:, :, 0, 0])        # [256,1024]
    w1b = np.ascontiguousarray(
        w1.T.reshape(4, 2, 128, 256).transpose(2, 0, 1, 3).astype(f8)
    )                                                                      # [128,4,2,256]
    w2 = np.sign(np.asarray(inputs["w2"], np.float32))                     # [256,256,3,3]
    w2l = (
        w2.transpose(1, 2, 3, 0)                                           # [ci,ky,kx,co]
        .reshape(2, 128, 9, 256)                                           # [k,ki,tap,co]
        .transpose(1, 2, 0, 3)
        .copy()
    )                                                                      # [128,9,2,256]
    w2l[:, :, 1, :] *= 2.0      # k=1 inputs arrive as +-0.5 (DVE-binarized)
    w2b = np.ascontiguousarray(w2l.astype(f8))
    w3 = np.sign(np.asarray(inputs["w3"], np.float32)[:, :, 0, 0])         # [1024,256]
    w3l = w3.T.reshape(2, 128, 1024).transpose(1, 0, 2).copy()             # [128,2,1024]
    w3l[:, 1, :] *= 2.0         # k=1 inputs arrive as +-0.5 (DVE-binarized)
    w3b = np.ascontiguousarray(w3l.astype(f8))

    sc1, sh1 = _bn_params(inputs["g1"], inputs["b1"], inputs["m1"], inputs["v1"])
    sc2, sh2 = _bn_params(inputs["g2"], inputs["b2"], inputs["m2"], inputs["v2"])
    sc3, sh3 = _bn_params(inputs["g3"], inputs["b3"], inputs["m3"], inputs["v3"])

    wb = np.concatenate(
        [w1b.reshape(128, -1), w2b.reshape(128, -1), w3b.reshape(128, -1)],
        axis=1,
    )
    thr1 = -(sh1[128:] / sc1[128:])[:, None]       # m=1 threshold, [128,1]
    thr2 = -(sh2[128:] / sc2[128:])[:, None]
    bnp = np.concatenate(
        [
            sc1.reshape(2, 128).T, sh1.reshape(2, 128).T,
            sc2.reshape(2, 128).T, sh2.reshape(2, 128).T,
            sc3.reshape(8, 128).T, sh3.reshape(8, 128).T,
            thr1, thr2,
        ],
        axis=1,
    ).astype(np.float32)
    common = {
        "wb": np.ascontiguousarray(wb),
        "bnp": np.ascontiguousarray(bnp),
    }

    # x -> [core, sb, img(4), m(8), p(128), px] -> device layouts
    xr6 = x.reshape(N_CORES, NSB, BSB, 8, 128, NPX)
    xsg = np.sign(xr6)
    # residual with BN3 shift pre-folded: xr' = x + sh3[channel]
    xrs = xr6 + sh3.reshape(8, 128)[None, None, None, :, :, None]
    in_maps = []
    for c in range(N_CORES):
        # [sb, img, m, p, px] -> [sb, p, m, (img px)]
        perm = xrs[c].transpose(0, 3, 2, 1, 4).reshape(NSB, 128, 8, COLS)
        xsc = xsg[c].transpose(0, 3, 2, 1, 4).reshape(NSB, 128, 8, COLS)
        in_maps.append({
            "xs": np.ascontiguousarray(
                xsc.reshape(NSB, 128, 4, 2, COLS).astype(f8)
            ),
            "xr": np.ascontiguousarray(perm.astype(bf)),
            **common,
        })
    return in_maps


def _assemble_output(results):
    """results: list of per-core dicts with 'yt' [NSB,128,8,COLS] bf16."""
    y = np.empty((N_CORES, NSB, BSB, 8, 128, NPX), np.float32)
    for c, r in enumerate(results):
        yt = np.asarray(r["yt"]).astype(np.float32)        # [sb,p,m,cols]
        y[c] = yt.reshape(NSB, 128, 8, BSB, NPX).transpose(0, 3, 2, 1, 4)
    return np.ascontiguousarray(y.reshape(B, CIN, 14, 14))


def _run(inputs, trace=False):
    from concourse.bass_utils import run_bass_kernel_spmd

    if "nc" not in _state:
        _state["nc"] = _build_nc()
    nc = _state["nc"]
    in_maps = _prep_inputs(inputs)
    res = run_bass_kernel_spmd(
        nc, in_maps, core_ids=list(range(N_CORES)), trace=trace
    )
    return _assemble_output(res.results), res


def kernel(**inputs):
    out, _ = _run(inputs, trace=False)
    return out
